# revision 1
# baseline (speedup 1.0000x reference)
"""Trainium2 Bass kernel for the CoAttention DNS/Image module.

Math notes (exact algebraic simplification of the reference):
  scores1[b,r,s] = s_img[b,r] + s_dns[b,s] + b_att1 ; softmax over s.
  The per-row constant s_img[b,r] (and b_att1) cancels in the softmax, so
  a1[b,r,:] == softmax(s_dns[b,:]) for every r. Hence
      att_dns[b,r,:] = softmax(s_dns[b]) @ dns[b]          (same for all r)
  Similarly scores2's softmax over j kills t_dns[b,i] and b_att2, so
      att_img[b,i,:] = softmax(t_img[b]) @ img[b]          (same for all i)
  Therefore W_img1, w_att1[:H], b_att1, W_dns2, w_att2[:H], b_att2 do not
  affect the outputs at all.  The remaining work per batch item:
      s_dns[s] = tanh(dns[b] @ W_dns1.T + b_dns1) @ w_att1[H:]
      t_img[j] = tanh(img[b] @ W_img2.T + b_img2) @ w_att2[H:]
  plus two tiny softmaxes and two weighted sums.

Distribution: pure data parallel over batch (64 items -> 8 items/core on 8
NeuronCores), no collectives.  Per core the two HxH projections dominate:
8 * 2*(256+196)*1024*1024 ~= 7.6 GFLOP, run on the PE array in fp32r.
"""

import os
import sys

import numpy as np

try:
    import concourse  # noqa: F401
except ImportError:  # fresh environment: fall back to the repo path
    sys.path.insert(0, "/opt/trn_rl_repo")

B, S, R, H = 64, 256, 196, 1024
NCORES = 8
BPC = B // NCORES        # batch items per core = 8
PAIRS = BPC // 2         # items are processed in pairs = 4
HC = H // 128            # 8 chunks of the feature dim
ND = 2 * S               # dns pair free width  = 512
NG = 2 * R               # img pair free width  = 392

_CACHE = {}


def _build_program(use_f32r=True, repeat=1, loop_n=0, tail="full"):
    import concourse.bacc as bacc
    import concourse.tile as tile
    from concourse import mybir
    from contextlib import ExitStack

    f32 = mybir.dt.float32
    mm_dt = mybir.dt.float32r if use_f32r else mybir.dt.float32
    Act = mybir.ActivationFunctionType
    Ax = mybir.AxisListType

    nc = bacc.Bacc("TRN2", target_bir_lowering=False, debug=False)

    # Tensors feeding the PE array are typed float32r (same 4-byte layout as
    # f32; the verifier requires fp32r matmul operands to be produced as
    # fp32r end-to-end).  Non-matmul consumers bitcast back to f32.
    dtT = nc.dram_tensor("dtT", (BPC, H, S), mm_dt, kind="ExternalInput").ap()
    gtT = nc.dram_tensor("gtT", (BPC, H, R), mm_dt, kind="ExternalInput").ap()
    dnsN = nc.dram_tensor("dnsN", (BPC, S, H), mm_dt, kind="ExternalInput").ap()
    imgN = nc.dram_tensor("imgN", (BPC, R, H), mm_dt, kind="ExternalInput").ap()
    w1t = nc.dram_tensor("w1t", (H, H), mm_dt, kind="ExternalInput").ap()
    w4t = nc.dram_tensor("w4t", (H, H), mm_dt, kind="ExternalInput").ap()
    bc1 = nc.dram_tensor("bc1", (128, HC), f32, kind="ExternalInput").ap()
    bc4 = nc.dram_tensor("bc4", (128, HC), f32, kind="ExternalInput").ap()
    wd1 = nc.dram_tensor("wd1", (128, HC), mm_dt, kind="ExternalInput").ap()
    wi2 = nc.dram_tensor("wi2", (128, HC), mm_dt, kind="ExternalInput").ap()
    ones = nc.dram_tensor("ones", (1, 128), mm_dt, kind="ExternalInput").ap()

    att_dns = nc.dram_tensor("att_dns", (BPC, R, H), f32, kind="ExternalOutput").ap()
    att_img = nc.dram_tensor("att_img", (BPC, R, H), f32, kind="ExternalOutput").ap()

    with tile.TileContext(nc) as tc, ExitStack() as ctx:
        consts = ctx.enter_context(tc.tile_pool(name="consts", bufs=1))
        acts = ctx.enter_context(tc.tile_pool(name="acts", bufs=2))
        nats = ctx.enter_context(tc.tile_pool(name="nats", bufs=2))
        tpool = ctx.enter_context(tc.tile_pool(name="tpool", bufs=3))
        smalls = ctx.enter_context(tc.tile_pool(name="smalls", bufs=2))
        arp_sb = ctx.enter_context(tc.tile_pool(name="arp_sb", bufs=4))
        obs = ctx.enter_context(tc.tile_pool(name="obs", bufs=2))
        pproj = ctx.enter_context(tc.tile_pool(name="pproj", bufs=4, space="PSUM"))
        psrow = ctx.enter_context(tc.tile_pool(name="psrow", bufs=2, space="PSUM"))
        pob = ctx.enter_context(tc.tile_pool(name="pob", bufs=1, space="PSUM"))

        # --- constants ---
        w1_sb = consts.tile([128, HC * H], mm_dt, name="w1_sb")
        nc.sync.dma_start(
            out=w1_sb.rearrange("p (hc o) -> p hc o", hc=HC),
            in_=w1t.rearrange("(hc p) o -> p hc o", p=128),
        )
        w4_sb = consts.tile([128, HC * H], mm_dt, name="w4_sb")
        nc.sync.dma_start(
            out=w4_sb.rearrange("p (hc o) -> p hc o", hc=HC),
            in_=w4t.rearrange("(hc p) o -> p hc o", p=128),
        )
        b1_sb = consts.tile([128, HC], f32, name="b1_sb")
        nc.sync.dma_start(out=b1_sb, in_=bc1)
        b4_sb = consts.tile([128, HC], f32, name="b4_sb")
        nc.sync.dma_start(out=b4_sb, in_=bc4)
        wd1_sb = consts.tile([128, HC], mm_dt, name="wd1_sb")
        nc.sync.dma_start(out=wd1_sb, in_=wd1)
        wi2_sb = consts.tile([128, HC], mm_dt, name="wi2_sb")
        nc.sync.dma_start(out=wi2_sb, in_=wi2)
        ones_sb = consts.tile([1, 128], mm_dt, name="ones_sb")
        nc.sync.dma_start(out=ones_sb, in_=ones)

        import contextlib
        loop_cm = (tc.For_i(0, loop_n, 1, hint_engines=(mybir.EngineType.PE,))
                   if loop_n else contextlib.nullcontext())
        with loop_cm:
         for rep in range(repeat):
          for pr in range(PAIRS):
            dt = acts.tile([128, HC * ND], mm_dt, tag="dt", name=f"dt{rep}_{pr}")
            gt = acts.tile([128, HC * NG], mm_dt, tag="gt", name=f"gt{rep}_{pr}")
            for j in (0, 1):
                it = 2 * pr + j
                nc.sync.dma_start(
                    out=dt.rearrange("p (hc j s) -> p hc j s", hc=HC, j=2)[:, :, j, :],
                    in_=dtT[it].rearrange("(hc p) s -> p hc s", p=128),
                )
                nc.sync.dma_start(
                    out=gt.rearrange("p (hc j s) -> p hc j s", hc=HC, j=2)[:, :, j, :],
                    in_=gtT[it].rearrange("(hc p) s -> p hc s", p=128),
                )
            # natural-layout tiles for the attention-weighted sums
            # free layout: (j, sc, h)
            dnat = nats.tile([128, 4 * H], mm_dt, tag="nat", name=f"dn{rep}_{pr}")
            gnat = nats.tile([128, 4 * H], mm_dt, tag="nat", name=f"gn{rep}_{pr}")
            for j in (0, 1):
                it = 2 * pr + j
                for sc in (0, 1):
                    nc.sync.dma_start(
                        out=dnat[:, (j * 2 + sc) * H:(j * 2 + sc + 1) * H],
                        in_=dnsN[it, sc * 128:(sc + 1) * 128, :])
                    rows = 128 if sc == 0 else R - 128
                    nc.sync.dma_start(
                        out=gnat[0:rows, (j * 2 + sc) * H:(j * 2 + sc + 1) * H],
                        in_=imgN[it, sc * 128:sc * 128 + rows, :])

            for side in (0, 1):
                if side == 0:
                    act_t, nat, w_sb, b_sb, wv_sb, n, ns, out_ap = (
                        dt, dnat, w1_sb, b1_sb, wd1_sb, ND, S, att_dns)
                else:
                    act_t, nat, w_sb, b_sb, wv_sb, n, ns, out_ap = (
                        gt, gnat, w4_sb, b4_sb, wi2_sb, NG, R, att_img)

                # srow[j*ns+s] = sum_o w[o] * tanh(proj[o, j*ns+s] + b[o])
                srow = psrow.tile([1, n], f32, tag="srow", name=f"srow{rep}_{pr}_{side}")
                for oc in range(HC):
                    pj = pproj.tile([128, n], f32, tag="proj", name=f"pj{rep}_{pr}_{side}_{oc}")
                    for hc in range(HC):
                        nc.tensor.matmul(
                            pj,
                            lhsT=w_sb[:, hc * H + oc * 128: hc * H + (oc + 1) * 128],
                            rhs=act_t[:, hc * n:(hc + 1) * n],
                            start=(hc == 0),
                            stop=(hc == HC - 1),
                        )
                    tt = tpool.tile([128, n], mm_dt, tag="T", name=f"tt{rep}_{pr}_{side}_{oc}")
                    nc.scalar.activation(
                        out=tt, in_=pj, func=Act.Tanh,
                        bias=b_sb[:, oc:oc + 1], scale=1.0,
                    )
                    nc.tensor.matmul(
                        srow,
                        lhsT=wv_sb[:, oc:oc + 1],
                        rhs=tt,
                        start=(oc == 0),
                        stop=(oc == HC - 1),
                    )

                if tail == "none":
                    if pr == 0 and rep == 0 and side == 0:
                        dump = smalls.tile([1, n], f32, tag="dump", name=f"du{rep}")
                        nc.vector.tensor_copy(out=dump, in_=srow)
                        nc.sync.dma_start(out=att_dns[0, 0:1, :n], in_=dump)
                    continue

                # softmax over each item's slice of srow.  The logits are
                # bounded (|s| <= sum|w| ~ 16) so max-subtraction is
                # unnecessary in fp32.
                e_pair = smalls.tile([1, n], f32, tag="e", name=f"e{rep}_{pr}_{side}")
                a_pair = smalls.tile([1, n], mm_dt, tag="a", name=f"a{rep}_{pr}_{side}")
                for j in (0, 1):
                    sl = srow[0:1, j * ns:(j + 1) * ns]
                    sm = smalls.tile([1, 1], f32, tag="sm", name=f"sm{rep}_{pr}_{side}_{j}")
                    nc.scalar.activation(
                        out=e_pair[0:1, j * ns:(j + 1) * ns], in_=sl,
                        func=Act.Exp, accum_out=sm,
                    )
                    rv = smalls.tile([1, 1], f32, tag="rv", name=f"rv{rep}_{pr}_{side}_{j}")
                    nc.vector.reciprocal(out=rv, in_=sm)
                    nc.vector.tensor_scalar_mul(
                        a_pair[0:1, j * ns:(j + 1) * ns],
                        e_pair[0:1, j * ns:(j + 1) * ns], rv)

                # att[it] rows are all  v = sum_s a[s] * X[s, :].  Build
                # a as a 128-wide stationary operand (outer product with
                # ones), then matmul against the natural-layout features:
                # every psum row of ob becomes v at once.
                for j in (0, 1):
                    it = 2 * pr + j
                    obp = pob.tile([128, H], f32, tag="ob", name=f"obp{rep}_{it}_{side}")
                    for sc in (0, 1):
                        w = min(128, ns - sc * 128)
                        arp = pproj.tile([128, 128], f32, tag="proj",
                                         name=f"arp{rep}_{it}_{side}_{sc}")
                        nc.tensor.matmul(
                            arp[0:w, :],
                            lhsT=a_pair[0:1, j * ns + sc * 128: j * ns + sc * 128 + w],
                            rhs=ones_sb,
                            start=True, stop=True)
                        ar = arp_sb.tile([128, 128], mm_dt, tag="ar",
                                         name=f"ar{rep}_{it}_{side}_{sc}")
                        nc.vector.tensor_copy(out=ar[0:w, :], in_=arp[0:w, :])
                        for hh in (0, 1):
                            nc.tensor.matmul(
                                obp[:, hh * 512:(hh + 1) * 512],
                                lhsT=ar[0:w, :],
                                rhs=nat[0:w, (j * 2 + sc) * H + hh * 512:
                                        (j * 2 + sc) * H + (hh + 1) * 512],
                                start=(sc == 0), stop=(sc == 1))
                    obc = obs.tile([128, H], f32, tag="obc", name=f"obc{rep}_{it}_{side}")
                    nc.scalar.copy(out=obc[:, 0:512], in_=obp[:, 0:512])
                    nc.vector.tensor_copy(out=obc[:, 512:1024], in_=obp[:, 512:1024])
                    # output stores ride the ACT HWDGE ring so the SP-ring
                    # input loads never queue behind them
                    nc.scalar.dma_start(out=out_ap[it, 0:128, :], in_=obc)
                    nc.scalar.dma_start(out=out_ap[it, 128:R, :], in_=obc[0:R - 128, :])

    nc.compile()
    return nc


def _get_program(repeat=1):
    key = ("prog", repeat)
    if key not in _CACHE:
        use_f32r = os.environ.get("COATT_F32R", "1") == "1"
        _CACHE[key] = _build_program(use_f32r=use_f32r, repeat=repeat)
    return _CACHE[key]


def _prepare_in_maps(dns_feature, img_features, W_dns1, b_dns1, W_img2, b_img2,
                     w_att1, w_att2):
    dns_nat = np.ascontiguousarray(np.asarray(dns_feature, np.float32))
    img_nat = np.ascontiguousarray(np.asarray(img_features, np.float32))
    dns = np.ascontiguousarray(dns_nat.transpose(0, 2, 1))
    img = np.ascontiguousarray(img_nat.transpose(0, 2, 1))
    w1t = np.ascontiguousarray(np.asarray(W_dns1, np.float32).T)
    w4t = np.ascontiguousarray(np.asarray(W_img2, np.float32).T)
    bc1 = np.ascontiguousarray(np.asarray(b_dns1, np.float32).reshape(HC, 128).T)
    bc4 = np.ascontiguousarray(np.asarray(b_img2, np.float32).reshape(HC, 128).T)
    wd1 = np.ascontiguousarray(np.asarray(w_att1, np.float32)[H:].reshape(HC, 128).T)
    wi2 = np.ascontiguousarray(np.asarray(w_att2, np.float32)[H:].reshape(HC, 128).T)
    ones = np.ones((1, 128), dtype=np.float32)
    in_maps = []
    for c in range(NCORES):
        in_maps.append({
            "dtT": np.ascontiguousarray(dns[c * BPC:(c + 1) * BPC]),
            "gtT": np.ascontiguousarray(img[c * BPC:(c + 1) * BPC]),
            "dnsN": np.ascontiguousarray(dns_nat[c * BPC:(c + 1) * BPC]),
            "imgN": np.ascontiguousarray(img_nat[c * BPC:(c + 1) * BPC]),
            "w1t": w1t, "w4t": w4t, "bc1": bc1, "bc4": bc4,
            "wd1": wd1, "wi2": wi2, "ones": ones,
        })
    return in_maps


def run(inputs, trace=False):
    """Run on the 8 NeuronCores; returns (att_img, att_dns, exec_time_ns)."""
    from concourse.bass_utils import run_bass_kernel_spmd

    nc = _get_program()
    in_maps = _prepare_in_maps(
        inputs["dns_feature"], inputs["img_features"],
        inputs["W_dns1"], inputs["b_dns1"], inputs["W_img2"], inputs["b_img2"],
        inputs["w_att1"], inputs["w_att2"],
    )
    res = run_bass_kernel_spmd(nc, in_maps, core_ids=list(range(NCORES)),
                               trace=trace)
    att_img = np.concatenate([res.results[c]["att_img"] for c in range(NCORES)], 0)
    att_dns = np.concatenate([res.results[c]["att_dns"] for c in range(NCORES)], 0)
    return att_img, att_dns, res.exec_time_ns


def kernel(**inputs):
    att_img, att_dns, _ = run(inputs, trace=False)
    return att_img, att_dns


if __name__ == "__main__":
    prog = _get_program()
    print("program built + compiled OK")

